# revision 29
# baseline (speedup 1.0000x reference)
"""Trainium2 Bass kernel for nn_AuxiliaryHybridRecurrentFFN.

Reference computation (B=4, S=2048, H=R=L=2048):
    f = tanh(x @ w_f);  g = sigmoid(x @ w_i) * silu(x @ w_v)
    states[t] = f[t] * states[t-1] + g[t]        (diagonal recurrence, s0 = 0)
    readout = silu((x @ w_q) * states)
    local = (x @ w_up * silu(x @ w_gate)) @ w_down
    out = readout @ w_ro + local
    aux_loss = mean((silu(states[:, :-1]) @ w_aux - x[:, 1:])**2)

Sharding: data-parallel over 8 shards = (batch, seq-half of 1024). Each core
additionally computes W=512 warmup timesteps before its shard so the scan can
start from zero: |tanh| forget gates contract the influence of the true
entry state by ~e^-350 over 512 steps, far below f32 resolution. For the
first half of each sequence the warmup inputs are zero-padded, which makes
f=g=0 and reproduces the zero initial state exactly.

On-device layout: everything is computed transposed (Y^T = W^T @ x^T with the
weight as the stationary matmul operand), so activations land as
[R-partitions, time-free] — the layout the native tensor_tensor_scan
(state = f*state + g along the free dim) requires. Matmuls run in float32r
(full-speed PE mode, TF32-like rounding). The aux-loss MSE is reduced
on-device to per-partition partial sums; the host sums them.
"""

import sys

sys.path.insert(0, "/opt/trn_rl_repo")

import ml_dtypes
import numpy as np

import concourse.bass as bass
import concourse.tile as tile
from concourse.tile import add_dep_helper
from concourse import bacc, mybir
from concourse.bass_utils import run_bass_kernel_spmd

P = 128
AF = mybir.ActivationFunctionType
ALU = mybir.AluOpType
F32 = mybir.dt.float32
F32R = mybir.dt.float32r
BF16 = mybir.dt.bfloat16


def build_nc(B=4, S=2048, H=2048, R=2048, L=2048, T=1024, W=512, C=512,
             relay=True):
    if relay:
        W = 0  # exact cross-core state relay replaces warmup
    nH, nR, nL = H // P, R // P, L // P
    TOT = W + T        # scan columns
    NCH = TOT // C     # scan chunks
    WCH = W // C       # warmup chunks
    NHF = T // C       # real chunks ("halves")
    XC = W + T + 1     # xt columns (one extra for the aux target shift)
    assert TOT % C == 0 and W % C == 0 and T % C == 0

    nc = bacc.Bacc("TRN2", target_bir_lowering=False, debug=False, num_devices=8)

    def din(name, shape):
        return nc.dram_tensor(name, shape, F32R, kind="ExternalInput")

    # weights are host-packed to [nM, P, K] so one output-tile slice is a
    # single contiguous row per partition (8 KB DMA descriptors)
    xt_d = din("xt", [H, XC])
    wf_d, wi_d, wv_d, wq_d = (
        din(n, [R // P, P, H]) for n in ("w_f", "w_i", "w_v", "w_q")
    )
    wup_d, wgate_d = din("w_up", [L // P, P, H]), din("w_gate", [L // P, P, H])
    wro_d = din("w_ro", [nH, P, R])
    waux_d = nc.dram_tensor("w_aux", [nH, P, R], BF16, kind="ExternalInput")
    wdn_d = din("w_down", [nH, P, L])
    hmask_d = (
        nc.dram_tensor("hmask", [P, 1], F32, kind="ExternalInput") if relay else None
    )
    outT_d = nc.dram_tensor("outT", [H, T], F32, kind="ExternalOutput")
    NSSE = nH * NHF + nH
    sse_d = nc.dram_tensor("sse", [P, NSSE], F32, kind="ExternalOutput")

    xt_r = xt_d.ap().rearrange("(k p) c -> p k c", p=P)
    outT_r = outT_d.ap().rearrange("(k p) c -> p k c", p=P)

    def wtile(d, m):  # packed weight, output-tile m -> [P, nK, P]
        return d.ap()[m].rearrange("p (k j) -> p k j", j=P)

    def vv(ap):  # f32 view for vector/scalar engine access to f32r data
        return ap.bitcast(F32)

    with tile.TileContext(nc) as tc:
        with (
            tc.tile_pool(name="ps", bufs=8, space="PSUM") as ps,
            tc.tile_pool(name="dramp", bufs=1, space="DRAM") as dramp,
            tc.tile_pool(name="misc", bufs=1) as misc,
        ):
            states_d = dramp.tile([P, nR, T], F32R)
            q_d = dramp.tile([P, nR, T], F32)
            sse_sb = misc.tile([P, NSSE], F32)
            if relay:
                cum_d = dramp.tile([P, nR, T], F32)
                final_d = dramp.tile([P, nR], F32)
                gath_d = dramp.tile([2, P, nR], F32)
                final_sb = misc.tile([P, nR], F32)
                mask_sb = misc.tile([P, 1], F32)
                nc.sync.dma_start(out=mask_sb, in_=hmask_d.ap())

            # ================= Stage A: f, i, v matmuls + scan =================
            xta_cm = tc.tile_pool(name="xta", bufs=1)
            w_cm = tc.tile_pool(name="w", bufs=4)
            xta_pool = xta_cm.__enter__()
            w_pool = w_cm.__enter__()
            with (
                tc.tile_pool(name="fg", bufs=3) as fg_pool,
                tc.tile_pool(name="stA", bufs=3) as stA_pool,
                tc.tile_pool(name="zA", bufs=1) as zA_pool,
            ):
                if relay:
                    zero_sb = zA_pool.tile([P, C], F32)
                    nc.vector.memset(zero_sb, 0.0)
                xt_sb = xta_pool.tile([P, nH, XC], F32R)
                # interleaved startup: wf0 k-slices + first-chunk x columns land
                # together so matmul (m=0,c=0,k) can start as slice k arrives
                wt_f = w_pool.tile([P, nH, P], F32R, tag="w", name="w0_f")
                for k in range(nH):
                    nc.sync.dma_start(out=wt_f[:, k, :], in_=wtile(wf_d, 0)[:, k, :])
                    nc.sync.dma_start(out=xt_sb[:, k, 0:C], in_=xt_r[:, k, 0:C])
                w0 = [wt_f]
                for d in (wi_d, wv_d):
                    wt = w_pool.tile([P, nH, P], F32R, tag="w", name=f"w0_{d.name}")
                    nc.sync.dma_start(out=wt, in_=wtile(d, 0))
                    w0.append(wt)
                for c in range(1, NCH):
                    for k in range(nH):
                        csl0 = slice(c * C, ((c + 1) * C + 1) if c == NCH - 1 else (c + 1) * C)
                        nc.sync.dma_start(out=xt_sb[:, k, csl0], in_=xt_r[:, k, csl0])

                for m in range(nR):
                    if m == 0:
                        wf_m, wi_m, wv_m = w0
                    else:
                        wf_m = w_pool.tile([P, nH, P], F32R, tag="w")
                        wi_m = w_pool.tile([P, nH, P], F32R, tag="w")
                        wv_m = w_pool.tile([P, nH, P], F32R, tag="w")
                        nc.sync.dma_start(out=wf_m, in_=wtile(wf_d, m))
                        nc.sync.dma_start(out=wi_m, in_=wtile(wi_d, m))
                        nc.sync.dma_start(out=wv_m, in_=wtile(wv_d, m))
                    wq_m = w_pool.tile([P, nH, P], F32R, tag="w")
                    nc.sync.dma_start(out=wq_m, in_=wtile(wq_d, m))
                    prev_last = None
                    for c in range(NCH):
                        csl = slice(c * C, (c + 1) * C)
                        psf = ps.tile([P, C], F32, tag="ps")
                        psi = ps.tile([P, C], F32, tag="ps")
                        psv = ps.tile([P, C], F32, tag="ps")
                        psq = ps.tile([P, C], F32, tag="ps")
                        for w_m, p_t in ((wf_m, psf), (wi_m, psi), (wv_m, psv),
                                         (wq_m, psq)):
                            for k in range(nH):
                                nc.tensor.matmul(
                                    p_t, w_m[:, k, :], xt_sb[:, k, csl],
                                    start=(k == 0), stop=(k == nH - 1),
                                )
                        q_t = fg_pool.tile([P, C], F32, tag="qv")
                        nc.scalar.copy(q_t, psq)
                        nc.sync.dma_start(
                            out=q_d[:, m, csl], in_=q_t
                        )
                        f_t = fg_pool.tile([P, C], F32, tag="f")
                        nc.scalar.activation(f_t, psf, AF.Tanh)
                        sig_t = fg_pool.tile([P, C], F32, tag="sig")
                        nc.scalar.activation(sig_t, psi, AF.Sigmoid)
                        vs_t = fg_pool.tile([P, C], F32, tag="vs")
                        nc.scalar.activation(vs_t, psv, AF.Sigmoid)
                        svl_t = fg_pool.tile([P, C], F32, tag="svl")
                        nc.vector.tensor_mul(svl_t, psv, vs_t)  # silu(Yv)
                        g_t = fg_pool.tile([P, C], F32, tag="g")
                        nc.vector.tensor_mul(g_t, sig_t, svl_t)
                        st_t = stA_pool.tile([P, C], F32R, tag="st")
                        nc.vector.tensor_tensor_scan(
                            st_t, f_t, g_t,
                            0.0 if c == 0 else prev_last,
                            op0=ALU.mult, op1=ALU.add,
                        )
                        if c >= WCH:
                            rsl = slice((c - WCH) * C, (c - WCH + 1) * C)
                            nc.sync.dma_start(out=states_d[:, m, rsl], in_=st_t)
                        prev_last = vv(st_t[:, C - 1 : C])
                        if relay:
                            cu_t = stA_pool.tile([P, C], F32, tag="cu")
                            nc.vector.tensor_tensor_scan(
                                cu_t, f_t, zero_sb,
                                1.0 if c == 0 else prev_cum,
                                op0=ALU.mult, op1=ALU.add,
                            )
                            nc.sync.dma_start(
                                out=cum_d[:, m, c * C : (c + 1) * C], in_=cu_t
                            )
                            prev_cum = cu_t[:, C - 1 : C]
                            if c == NCH - 1:
                                nc.vector.tensor_copy(
                                    final_sb[:, m : m + 1], vv(st_t[:, C - 1 : C])
                                )

            if relay:
                nc.sync.dma_start(out=final_d[:, :], in_=final_sb)
                nc.gpsimd.collective_compute(
                    "AllGather", ALU.bypass,
                    replica_groups=[[0, 1], [2, 3], [4, 5], [6, 7]],
                    ins=[final_d[:, :]],
                    outs=[gath_d[:, :, :]],
                )
                sent_sb = misc.tile([P, nR], F32)
                nc.sync.dma_start(out=sent_sb, in_=gath_d[0])
                # entry state = (partner's final state) * per-core mask
                nc.vector.tensor_scalar_mul(sent_sb, sent_sb, mask_sb[:, 0:1])

            # ============ Stage B: q/up/gate, readout, out, aux ============
            with (
                tc.tile_pool(name="sth", bufs=1) as sth_pool,
                tc.tile_pool(name="rop", bufs=1) as ro_pool,
                tc.tile_pool(name="hp", bufs=1) as h_pool,
                tc.tile_pool(name="bt", bufs=2) as bt_pool,
            ):
                out_pool = bt_pool
                wB_pool = w_pool
                psa_last = []
                for hf in range(NHF):
                    base = W + hf * C
                    xt_h = xt_sb[:, :, base : base + C + 1]
                    prev_psa, psa_last = psa_last, []
                    st_h = sth_pool.tile([P, nR, C], F32R, tag="sth")
                    for k in range(nR):
                        dmi = nc.sync.dma_start(
                            out=st_h[:, k, :],
                            in_=states_d[:, k, hf * C : (hf + 1) * C],
                        )
                        for pl in prev_psa:
                            add_dep_helper(dmi.ins, pl.ins,
                                           reason="st_h reload after aux reads")
                    ro_t = ro_pool.tile([P, nR, C], F32R, tag="ro")
                    h_t = h_pool.tile([P, nL, C], F32R, tag="h")

                    # ---- up/gate phase: h = up * silu(gate) ----
                    for m in range(nL):
                        wu_m = wB_pool.tile([P, nH, P], F32R, tag="w")
                        wg_m = wB_pool.tile([P, nH, P], F32R, tag="w")
                        nc.sync.dma_start(out=wu_m, in_=wtile(wup_d, m))
                        nc.sync.dma_start(out=wg_m, in_=wtile(wgate_d, m))
                        psu = ps.tile([P, C], F32, tag="ps")
                        psg = ps.tile([P, C], F32, tag="ps")
                        for k in range(nH):
                            nc.tensor.matmul(
                                psu, wu_m[:, k, :], xt_h[:, k, 0:C],
                                start=(k == 0), stop=(k == nH - 1),
                            )
                        for k in range(nH):
                            nc.tensor.matmul(
                                psg, wg_m[:, k, :], xt_h[:, k, 0:C],
                                start=(k == 0), stop=(k == nH - 1),
                            )
                        t2 = bt_pool.tile([P, C], F32, tag="tt")
                        nc.scalar.activation(t2, psg, AF.Sigmoid)
                        t1 = bt_pool.tile([P, C], F32, tag="tt")
                        nc.vector.tensor_mul(t1, psg, t2)  # silu(gate)
                        nc.vector.tensor_mul(h_t[:, m, :], psu, t1)

                    if relay:
                        for k2 in range(nR):
                            cu_k = bt_pool.tile([P, C], F32, tag="cu", bufs=1)
                            nc.sync.dma_start(
                                out=cu_k,
                                in_=cum_d[:, k2, hf * C : (hf + 1) * C],
                            )
                            nc.vector.scalar_tensor_tensor(
                                st_h[:, k2, :], cu_k,
                                sent_sb[:, k2 : k2 + 1], vv(st_h[:, k2, :]),
                                op0=ALU.mult, op1=ALU.add,
                            )

                    # ---- q phase: readout = silu(q * states); q from spill ----
                    q_muls = []
                    for m in range(nR):
                        q_m = bt_pool.tile([P, C], F32, tag="qh", bufs=3)
                        nc.sync.dma_start(out=q_m, in_=q_d[:, m, hf * C : (hf + 1) * C])
                        t1 = bt_pool.tile([P, C], F32, tag="tt")
                        q_muls.append(nc.vector.tensor_mul(t1, q_m, vv(st_h[:, m, :])))
                        t2 = bt_pool.tile([P, C], F32, tag="tt")
                        nc.scalar.activation(t2, t1, AF.Sigmoid)
                        nc.vector.tensor_mul(ro_t[:, m, :], t1, t2)

                    # ---- out = readout @ w_ro + h @ w_down; aux fused in ----
                    for m2 in range(nH):
                        wro_m = wB_pool.tile([P, nR, P], F32R, tag="w")
                        wdn_m = wB_pool.tile([P, nL, P], F32R, tag="w")
                        nc.sync.dma_start(out=wro_m, in_=wtile(wro_d, m2))
                        nc.sync.dma_start(out=wdn_m, in_=wtile(wdn_d, m2))
                        pso = ps.tile([P, C], F32, tag="ps")
                        for k2 in range(nR):
                            nc.tensor.matmul(
                                pso, wro_m[:, k2, :], ro_t[:, k2, :],
                                start=(k2 == 0), stop=False,
                            )
                        for k2 in range(nL):
                            nc.tensor.matmul(
                                pso, wdn_m[:, k2, :], h_t[:, k2, :],
                                start=False, stop=(k2 == nL - 1),
                            )
                        o_t = out_pool.tile([P, C], F32, tag="tt")
                        nc.scalar.copy(o_t, pso)
                        nc.sync.dma_start(
                            out=outT_r[:, m2, hf * C : (hf + 1) * C], in_=o_t
                        )
                    # ---- ss = silu(states) -> bf16, written in place over the
                    # f32 states row via a bitcast view (write addr trails read addr)
                    st_bf = st_h.bitcast(BF16)
                    ss_muls = []
                    for k2 in range(nR):
                        t2 = bt_pool.tile([P, C], F32, tag="tt")
                        nc.scalar.activation(t2, vv(st_h[:, k2, :]), AF.Sigmoid)
                        ssm = nc.vector.tensor_mul(
                            st_bf[:, k2, 0:C], vv(st_h[:, k2, :]), t2
                        )
                        add_dep_helper(ssm.ins, q_muls[k2].ins,
                                       reason="ss bf16 write after q read (bitcast)")
                        ss_muls.append(ssm)

                    for m2 in range(nH):
                        wax_m = wB_pool.tile([P, nR, P], BF16, tag="w")
                        nc.sync.dma_start(out=wax_m, in_=wtile(waux_d, m2))
                        psa = ps.tile([P, C], F32, tag="ps")
                        for k2 in range(nR):
                            mmi = nc.tensor.matmul(
                                psa, wax_m[:, k2, :], st_bf[:, k2, 0:C],
                                start=(k2 == 0), stop=(k2 == nR - 1),
                            )
                            add_dep_helper(mmi.ins, ss_muls[k2].ins,
                                           reason="aux mm after ss write (bitcast)")
                            if k2 == nR - 1:
                                psa_last.append(mmi)
                        err = bt_pool.tile([P, C], F32, tag="tt")
                        nc.vector.tensor_sub(err, psa, vv(xt_h[:, m2, 1 : C + 1]))
                        if hf < NHF - 1:
                            nc.scalar.activation(
                                err, err, AF.Square,
                                accum_out=sse_sb[:, m2 * NHF + hf : m2 * NHF + hf + 1],
                            )
                        else:
                            nc.scalar.activation(
                                err[:, 0 : C - 1], err[:, 0 : C - 1], AF.Square,
                                accum_out=sse_sb[:, m2 * NHF + hf : m2 * NHF + hf + 1],
                            )
                            nc.scalar.activation(
                                err[:, C - 1 : C], err[:, C - 1 : C], AF.Square,
                                accum_out=sse_sb[:, nH * NHF + m2 : nH * NHF + m2 + 1],
                            )

            nc.sync.dma_start(out=sse_d.ap(), in_=sse_sb)
            w_cm.__exit__(None, None, None)
            xta_cm.__exit__(None, None, None)

    nc.compile()
    return nc


_NC_CACHE = {}
PROFILE = False
LAST_EXEC_NS = None


def _get_nc(key, **kw):
    if key not in _NC_CACHE:
        _NC_CACHE[key] = build_nc(**kw)
    return _NC_CACHE[key]


def kernel(x, w_f, w_i, w_v, w_q, w_ro, w_aux, w_up, w_gate, w_down, init_state):
    x = np.asarray(x, np.float32)
    def pack(w):  # [K, M] -> [nM, P, K]; tile m contiguous per partition
        w = np.asarray(w, np.float32)
        K, M = w.shape
        return np.ascontiguousarray(
            w.reshape(K // P, P, M // P, P).transpose(2, 1, 0, 3).reshape(M // P, P, K)
        )

    ws = {
        "w_f": pack(w_f), "w_i": pack(w_i), "w_v": pack(w_v), "w_q": pack(w_q),
        "w_up": pack(w_up), "w_gate": pack(w_gate),
        "w_ro": pack(w_ro), "w_down": pack(w_down),
        "w_aux": pack(w_aux).astype(ml_dtypes.bfloat16),
    }
    B, S, H = x.shape
    T = S // 2
    C = T // 2
    W = 0
    XC = W + T + 1
    nc = _get_nc((B, S, H), B=B, S=S, H=H, R=H, L=H, T=T, W=W, C=C, relay=True)

    # 8 shards: (batch, half). Shard i -> b = i // 2, hf = i % 2, t0 = hf*T.
    in_maps = []
    shard_meta = []
    for b in range(B):
        xTb = np.ascontiguousarray(x[b].T)  # [H, S]
        for hf in range(2):
            t0 = hf * T
            xt_aug = np.zeros((H, XC), np.float32)
            ta = max(0, t0 - W)            # first valid global t
            tb = min(S - 1, t0 + T)        # last valid global t
            j0 = ta - (t0 - W)
            xt_aug[:, j0 : j0 + (tb - ta + 1)] = xTb[:, ta : tb + 1]
            hmask = np.full((P, 1), 1.0 if hf == 1 else 0.0, np.float32)
            in_maps.append({"xt": xt_aug, "hmask": hmask, **ws})
            shard_meta.append((b, t0, hf))

    core_ids = list(range(8))
    res = run_bass_kernel_spmd(nc, in_maps, core_ids, trace=PROFILE)
    if PROFILE:
        global LAST_EXEC_NS
        LAST_EXEC_NS = res.exec_time_ns

    out = np.empty((B, S, H), np.float32)
    sse_total = 0.0
    for i, (b, t0, hf) in enumerate(shard_meta):
        r = res.results[i]
        out[b, t0 : t0 + T, :] = r["outT"].T
        sse = r["sse"]
        nH = H // P
        NHF = T // C
        sse_total += float(sse[:, : nH * NHF].sum())
        if t0 + T <= S - 1:  # last aux position t0+T-1 <= S-2 -> include
            sse_total += float(sse[:, nH * NHF :].sum())
    aux_loss = np.float32(sse_total / (B * (S - 1) * H))
    return out, aux_loss


# revision 30
# speedup vs baseline: 1.0831x; 1.0831x over previous
"""Trainium2 Bass kernel for nn_AuxiliaryHybridRecurrentFFN.

Reference computation (B=4, S=2048, H=R=L=2048):
    f = tanh(x @ w_f);  g = sigmoid(x @ w_i) * silu(x @ w_v)
    states[t] = f[t] * states[t-1] + g[t]        (diagonal recurrence, s0 = 0)
    readout = silu((x @ w_q) * states)
    local = (x @ w_up * silu(x @ w_gate)) @ w_down
    out = readout @ w_ro + local
    aux_loss = mean((silu(states[:, :-1]) @ w_aux - x[:, 1:])**2)

Sharding: data-parallel over 8 shards = (batch, seq-half of 1024). Each core
additionally computes W=512 warmup timesteps before its shard so the scan can
start from zero: |tanh| forget gates contract the influence of the true
entry state by ~e^-350 over 512 steps, far below f32 resolution. For the
first half of each sequence the warmup inputs are zero-padded, which makes
f=g=0 and reproduces the zero initial state exactly.

On-device layout: everything is computed transposed (Y^T = W^T @ x^T with the
weight as the stationary matmul operand), so activations land as
[R-partitions, time-free] — the layout the native tensor_tensor_scan
(state = f*state + g along the free dim) requires. Matmuls run in float32r
(full-speed PE mode, TF32-like rounding). The aux-loss MSE is reduced
on-device to per-partition partial sums; the host sums them.
"""

import sys

sys.path.insert(0, "/opt/trn_rl_repo")

import ml_dtypes
import numpy as np

import concourse.bass as bass
import concourse.tile as tile
from concourse.tile import add_dep_helper
from concourse import bacc, mybir
from concourse.bass_utils import run_bass_kernel_spmd

P = 128
AF = mybir.ActivationFunctionType
ALU = mybir.AluOpType
F32 = mybir.dt.float32
F32R = mybir.dt.float32r
BF16 = mybir.dt.bfloat16


def build_nc(B=4, S=2048, H=2048, R=2048, L=2048, T=1024, W=512, C=512,
             relay=True):
    if relay:
        W = 0  # exact cross-core state relay replaces warmup
    nH, nR, nL = H // P, R // P, L // P
    TOT = W + T        # scan columns
    NCH = TOT // C     # scan chunks
    WCH = W // C       # warmup chunks
    NHF = T // C       # real chunks ("halves")
    XC = W + T + 1     # xt columns (one extra for the aux target shift)
    assert TOT % C == 0 and W % C == 0 and T % C == 0

    nc = bacc.Bacc("TRN2", target_bir_lowering=False, debug=False, num_devices=8)

    def din(name, shape):
        return nc.dram_tensor(name, shape, F32R, kind="ExternalInput")

    # weights are host-packed to [nM, P, K] so one output-tile slice is a
    # single contiguous row per partition (8 KB DMA descriptors)
    xt_d = din("xt", [H, XC])
    wf_d, wi_d, wv_d, wq_d = (
        din(n, [R // P, P, H]) for n in ("w_f", "w_i", "w_v", "w_q")
    )
    wup_d, wgate_d = din("w_up", [L // P, P, H]), din("w_gate", [L // P, P, H])
    wro_d = din("w_ro", [nH, P, R])
    waux_d = nc.dram_tensor("w_aux", [nH, P, R], BF16, kind="ExternalInput")
    wdn_d = din("w_down", [nH, P, L])
    hmask_d = (
        nc.dram_tensor("hmask", [P, 1], F32, kind="ExternalInput") if relay else None
    )
    outT_d = nc.dram_tensor("outT", [H, T], F32, kind="ExternalOutput")
    NSSE = nH * NHF + nH
    sse_d = nc.dram_tensor("sse", [P, NSSE], F32, kind="ExternalOutput")

    xt_r = xt_d.ap().rearrange("(k p) c -> p k c", p=P)
    outT_r = outT_d.ap().rearrange("(k p) c -> p k c", p=P)

    def wtile(d, m):  # packed weight, output-tile m -> [P, nK, P]
        return d.ap()[m].rearrange("p (k j) -> p k j", j=P)

    def vv(ap):  # f32 view for vector/scalar engine access to f32r data
        return ap.bitcast(F32)

    with tile.TileContext(nc) as tc:
        with (
            tc.tile_pool(name="ps", bufs=6, space="PSUM") as ps,
            tc.tile_pool(name="dramp", bufs=1, space="DRAM") as dramp,
            tc.tile_pool(name="misc", bufs=1) as misc,
        ):
            states_d = dramp.tile([P, nR, T], F32R)
            sse_sb = misc.tile([P, NSSE], F32)
            if relay:
                cum_d = dramp.tile([P, nR, T], F32)
                final_d = dramp.tile([P, nR], F32)
                gath_d = dramp.tile([2, P, nR], F32)
                final_sb = misc.tile([P, nR], F32)
                mask_sb = misc.tile([P, 1], F32)
                nc.sync.dma_start(out=mask_sb, in_=hmask_d.ap())

            # ================= Stage A: f, i, v matmuls + scan =================
            xta_cm = tc.tile_pool(name="xta", bufs=1)
            w_cm = tc.tile_pool(name="w", bufs=5)
            xta_pool = xta_cm.__enter__()
            w_pool = w_cm.__enter__()
            with (
                tc.tile_pool(name="fg", bufs=3) as fg_pool,
                tc.tile_pool(name="stA", bufs=3) as stA_pool,
                tc.tile_pool(name="zA", bufs=1) as zA_pool,
            ):
                if relay:
                    zero_sb = zA_pool.tile([P, C], F32)
                    nc.vector.memset(zero_sb, 0.0)
                xt_sb = xta_pool.tile([P, nH, XC], F32R)
                # interleaved startup: wf0 k-slices + first-chunk x columns land
                # together so matmul (m=0,c=0,k) can start as slice k arrives
                wt_f = w_pool.tile([P, nH, P], F32R, tag="w", name="w0_f")
                for k in range(nH):
                    nc.sync.dma_start(out=wt_f[:, k, :], in_=wtile(wf_d, 0)[:, k, :])
                    nc.sync.dma_start(out=xt_sb[:, k, 0:C], in_=xt_r[:, k, 0:C])
                w0 = [wt_f]
                for d in (wi_d, wv_d):
                    wt = w_pool.tile([P, nH, P], F32R, tag="w", name=f"w0_{d.name}")
                    nc.sync.dma_start(out=wt, in_=wtile(d, 0))
                    w0.append(wt)
                for c in range(1, NCH):
                    for k in range(nH):
                        csl0 = slice(c * C, ((c + 1) * C + 1) if c == NCH - 1 else (c + 1) * C)
                        nc.sync.dma_start(out=xt_sb[:, k, csl0], in_=xt_r[:, k, csl0])

                for m in range(nR):
                    if m == 0:
                        wf_m, wi_m, wv_m = w0
                    else:
                        wf_m = w_pool.tile([P, nH, P], F32R, tag="w")
                        wi_m = w_pool.tile([P, nH, P], F32R, tag="w")
                        wv_m = w_pool.tile([P, nH, P], F32R, tag="w")
                        nc.sync.dma_start(out=wf_m, in_=wtile(wf_d, m))
                        nc.sync.dma_start(out=wi_m, in_=wtile(wi_d, m))
                        nc.sync.dma_start(out=wv_m, in_=wtile(wv_d, m))
                    prev_last = None
                    for c in range(NCH):
                        csl = slice(c * C, (c + 1) * C)
                        psf = ps.tile([P, C], F32, tag="ps")
                        psi = ps.tile([P, C], F32, tag="ps")
                        psv = ps.tile([P, C], F32, tag="ps")
                        for w_m, p_t in ((wf_m, psf), (wi_m, psi), (wv_m, psv)):
                            for k in range(nH):
                                nc.tensor.matmul(
                                    p_t, w_m[:, k, :], xt_sb[:, k, csl],
                                    start=(k == 0), stop=(k == nH - 1),
                                )
                        f_t = fg_pool.tile([P, C], F32, tag="f")
                        nc.scalar.activation(f_t, psf, AF.Tanh)
                        sig_t = fg_pool.tile([P, C], F32, tag="sig")
                        nc.scalar.activation(sig_t, psi, AF.Sigmoid)
                        vs_t = fg_pool.tile([P, C], F32, tag="vs")
                        nc.scalar.activation(vs_t, psv, AF.Sigmoid)
                        svl_t = fg_pool.tile([P, C], F32, tag="svl")
                        nc.vector.tensor_mul(svl_t, psv, vs_t)  # silu(Yv)
                        g_t = fg_pool.tile([P, C], F32, tag="g")
                        nc.vector.tensor_mul(g_t, sig_t, svl_t)
                        st_t = stA_pool.tile([P, C], F32R, tag="st")
                        nc.vector.tensor_tensor_scan(
                            st_t, f_t, g_t,
                            0.0 if c == 0 else prev_last,
                            op0=ALU.mult, op1=ALU.add,
                        )
                        if c >= WCH:
                            rsl = slice((c - WCH) * C, (c - WCH + 1) * C)
                            nc.sync.dma_start(out=states_d[:, m, rsl], in_=st_t)
                        prev_last = vv(st_t[:, C - 1 : C])
                        if relay:
                            cu_t = stA_pool.tile([P, C], F32, tag="cu")
                            nc.vector.tensor_tensor_scan(
                                cu_t, f_t, zero_sb,
                                1.0 if c == 0 else prev_cum,
                                op0=ALU.mult, op1=ALU.add,
                            )
                            nc.sync.dma_start(
                                out=cum_d[:, m, c * C : (c + 1) * C], in_=cu_t
                            )
                            prev_cum = cu_t[:, C - 1 : C]
                            if c == NCH - 1:
                                nc.vector.tensor_copy(
                                    final_sb[:, m : m + 1], vv(st_t[:, C - 1 : C])
                                )

            if relay:
                nc.sync.dma_start(out=final_d[:, :], in_=final_sb)
                nc.gpsimd.collective_compute(
                    "AllGather", ALU.bypass,
                    replica_groups=[[0, 1], [2, 3], [4, 5], [6, 7]],
                    ins=[final_d[:, :]],
                    outs=[gath_d[:, :, :]],
                )
                sent_sb = misc.tile([P, nR], F32)
                nc.sync.dma_start(out=sent_sb, in_=gath_d[0])
                # entry state = (partner's final state) * per-core mask
                nc.vector.tensor_scalar_mul(sent_sb, sent_sb, mask_sb[:, 0:1])

            # ============ Stage B: q/up/gate, readout, out, aux ============
            with (
                tc.tile_pool(name="sth", bufs=1) as sth_pool,
                tc.tile_pool(name="rop", bufs=1) as ro_pool,
                tc.tile_pool(name="hp", bufs=1) as h_pool,
                tc.tile_pool(name="bt", bufs=2) as bt_pool,
            ):
                out_pool = bt_pool
                wB_pool = w_pool
                psa_last = []
                for hf in range(NHF):
                    base = W + hf * C
                    xt_h = xt_sb[:, :, base : base + C + 1]
                    prev_psa, psa_last = psa_last, []
                    st_h = sth_pool.tile([P, nR, C], F32R, tag="sth")
                    for k in range(nR):
                        dmi = nc.sync.dma_start(
                            out=st_h[:, k, :],
                            in_=states_d[:, k, hf * C : (hf + 1) * C],
                        )
                        for pl in prev_psa:
                            add_dep_helper(dmi.ins, pl.ins,
                                           reason="st_h reload after aux reads")
                    ro_t = ro_pool.tile([P, nR, C], F32R, tag="ro")
                    h_t = h_pool.tile([P, nL, C], F32R, tag="h")

                    # ---- up/gate phase: h = up * silu(gate) ----
                    for m in range(nL):
                        wu_m = wB_pool.tile([P, nH, P], F32R, tag="w")
                        wg_m = wB_pool.tile([P, nH, P], F32R, tag="w")
                        nc.sync.dma_start(out=wu_m, in_=wtile(wup_d, m))
                        nc.sync.dma_start(out=wg_m, in_=wtile(wgate_d, m))
                        psu = ps.tile([P, C], F32, tag="ps")
                        psg = ps.tile([P, C], F32, tag="ps")
                        for k in range(nH):
                            nc.tensor.matmul(
                                psu, wu_m[:, k, :], xt_h[:, k, 0:C],
                                start=(k == 0), stop=(k == nH - 1),
                            )
                        for k in range(nH):
                            nc.tensor.matmul(
                                psg, wg_m[:, k, :], xt_h[:, k, 0:C],
                                start=(k == 0), stop=(k == nH - 1),
                            )
                        t2 = bt_pool.tile([P, C], F32, tag="tt")
                        nc.scalar.activation(t2, psg, AF.Sigmoid)
                        t1 = bt_pool.tile([P, C], F32, tag="tt")
                        nc.vector.tensor_mul(t1, psg, t2)  # silu(gate)
                        nc.vector.tensor_mul(h_t[:, m, :], psu, t1)

                    if relay:
                        for k2 in range(nR):
                            cu_k = bt_pool.tile([P, C], F32, tag="cu", bufs=1)
                            nc.sync.dma_start(
                                out=cu_k,
                                in_=cum_d[:, k2, hf * C : (hf + 1) * C],
                            )
                            nc.vector.scalar_tensor_tensor(
                                st_h[:, k2, :], cu_k,
                                sent_sb[:, k2 : k2 + 1], vv(st_h[:, k2, :]),
                                op0=ALU.mult, op1=ALU.add,
                            )

                    # ---- q phase: readout = silu(q * states) ----
                    q_muls = []
                    for m in range(nR):
                        wq_m = wB_pool.tile([P, nH, P], F32R, tag="w")
                        nc.sync.dma_start(out=wq_m, in_=wtile(wq_d, m))
                        psq = ps.tile([P, C], F32, tag="ps")
                        for k in range(nH):
                            nc.tensor.matmul(
                                psq, wq_m[:, k, :], xt_h[:, k, 0:C],
                                start=(k == 0), stop=(k == nH - 1),
                            )
                        t1 = bt_pool.tile([P, C], F32, tag="tt")
                        q_muls.append(nc.vector.tensor_mul(t1, psq, vv(st_h[:, m, :])))
                        t2 = bt_pool.tile([P, C], F32, tag="tt")
                        nc.scalar.activation(t2, t1, AF.Sigmoid)
                        nc.vector.tensor_mul(ro_t[:, m, :], t1, t2)

                    # ---- out = readout @ w_ro + h @ w_down; aux fused in ----
                    for m2 in range(nH):
                        wro_m = wB_pool.tile([P, nR, P], F32R, tag="w")
                        wdn_m = wB_pool.tile([P, nL, P], F32R, tag="w")
                        nc.sync.dma_start(out=wro_m, in_=wtile(wro_d, m2))
                        nc.sync.dma_start(out=wdn_m, in_=wtile(wdn_d, m2))
                        pso = ps.tile([P, C], F32, tag="ps")
                        for k2 in range(nR):
                            nc.tensor.matmul(
                                pso, wro_m[:, k2, :], ro_t[:, k2, :],
                                start=(k2 == 0), stop=False,
                            )
                        for k2 in range(nL):
                            nc.tensor.matmul(
                                pso, wdn_m[:, k2, :], h_t[:, k2, :],
                                start=False, stop=(k2 == nL - 1),
                            )
                        o_t = out_pool.tile([P, C], F32, tag="tt")
                        nc.scalar.copy(o_t, pso)
                        nc.sync.dma_start(
                            out=outT_r[:, m2, hf * C : (hf + 1) * C], in_=o_t
                        )
                    # ---- ss = silu(states) -> bf16, written in place over the
                    # f32 states row via a bitcast view (write addr trails read addr)
                    st_bf = st_h.bitcast(BF16)
                    ss_muls = []
                    for k2 in range(nR):
                        t2 = bt_pool.tile([P, C], F32, tag="tt")
                        nc.scalar.activation(t2, vv(st_h[:, k2, :]), AF.Sigmoid)
                        ssm = nc.vector.tensor_mul(
                            st_bf[:, k2, 0:C], vv(st_h[:, k2, :]), t2
                        )
                        add_dep_helper(ssm.ins, q_muls[k2].ins,
                                       reason="ss bf16 write after q read (bitcast)")
                        ss_muls.append(ssm)

                    for m2 in range(nH):
                        wax_m = wB_pool.tile([P, nR, P], BF16, tag="w")
                        nc.sync.dma_start(out=wax_m, in_=wtile(waux_d, m2))
                        psa = ps.tile([P, C], F32, tag="ps")
                        for k2 in range(nR):
                            mmi = nc.tensor.matmul(
                                psa, wax_m[:, k2, :], st_bf[:, k2, 0:C],
                                start=(k2 == 0), stop=(k2 == nR - 1),
                            )
                            add_dep_helper(mmi.ins, ss_muls[k2].ins,
                                           reason="aux mm after ss write (bitcast)")
                            if k2 == nR - 1:
                                psa_last.append(mmi)
                        err = bt_pool.tile([P, C], F32, tag="tt")
                        nc.vector.tensor_sub(err, psa, vv(xt_h[:, m2, 1 : C + 1]))
                        if hf < NHF - 1:
                            nc.scalar.activation(
                                err, err, AF.Square,
                                accum_out=sse_sb[:, m2 * NHF + hf : m2 * NHF + hf + 1],
                            )
                        else:
                            nc.scalar.activation(
                                err[:, 0 : C - 1], err[:, 0 : C - 1], AF.Square,
                                accum_out=sse_sb[:, m2 * NHF + hf : m2 * NHF + hf + 1],
                            )
                            nc.scalar.activation(
                                err[:, C - 1 : C], err[:, C - 1 : C], AF.Square,
                                accum_out=sse_sb[:, nH * NHF + m2 : nH * NHF + m2 + 1],
                            )

            nc.sync.dma_start(out=sse_d.ap(), in_=sse_sb)
            w_cm.__exit__(None, None, None)
            xta_cm.__exit__(None, None, None)

    nc.compile()
    return nc


_NC_CACHE = {}
PROFILE = False
LAST_EXEC_NS = None


def _get_nc(key, **kw):
    if key not in _NC_CACHE:
        _NC_CACHE[key] = build_nc(**kw)
    return _NC_CACHE[key]


def kernel(x, w_f, w_i, w_v, w_q, w_ro, w_aux, w_up, w_gate, w_down, init_state):
    x = np.asarray(x, np.float32)
    def pack(w):  # [K, M] -> [nM, P, K]; tile m contiguous per partition
        w = np.asarray(w, np.float32)
        K, M = w.shape
        return np.ascontiguousarray(
            w.reshape(K // P, P, M // P, P).transpose(2, 1, 0, 3).reshape(M // P, P, K)
        )

    ws = {
        "w_f": pack(w_f), "w_i": pack(w_i), "w_v": pack(w_v), "w_q": pack(w_q),
        "w_up": pack(w_up), "w_gate": pack(w_gate),
        "w_ro": pack(w_ro), "w_down": pack(w_down),
        "w_aux": pack(w_aux).astype(ml_dtypes.bfloat16),
    }
    B, S, H = x.shape
    T = S // 2
    C = T // 2
    W = 0
    XC = W + T + 1
    nc = _get_nc((B, S, H), B=B, S=S, H=H, R=H, L=H, T=T, W=W, C=C, relay=True)

    # 8 shards: (batch, half). Shard i -> b = i // 2, hf = i % 2, t0 = hf*T.
    in_maps = []
    shard_meta = []
    for b in range(B):
        xTb = np.ascontiguousarray(x[b].T)  # [H, S]
        for hf in range(2):
            t0 = hf * T
            xt_aug = np.zeros((H, XC), np.float32)
            ta = max(0, t0 - W)            # first valid global t
            tb = min(S - 1, t0 + T)        # last valid global t
            j0 = ta - (t0 - W)
            xt_aug[:, j0 : j0 + (tb - ta + 1)] = xTb[:, ta : tb + 1]
            hmask = np.full((P, 1), 1.0 if hf == 1 else 0.0, np.float32)
            in_maps.append({"xt": xt_aug, "hmask": hmask, **ws})
            shard_meta.append((b, t0, hf))

    core_ids = list(range(8))
    res = run_bass_kernel_spmd(nc, in_maps, core_ids, trace=PROFILE)
    if PROFILE:
        global LAST_EXEC_NS
        LAST_EXEC_NS = res.exec_time_ns

    out = np.empty((B, S, H), np.float32)
    sse_total = 0.0
    for i, (b, t0, hf) in enumerate(shard_meta):
        r = res.results[i]
        out[b, t0 : t0 + T, :] = r["outT"].T
        sse = r["sse"]
        nH = H // P
        NHF = T // C
        sse_total += float(sse[:, : nH * NHF].sum())
        if t0 + T <= S - 1:  # last aux position t0+T-1 <= S-2 -> include
            sse_total += float(sse[:, nH * NHF :].sum())
    aux_loss = np.float32(sse_total / (B * (S - 1) * H))
    return out, aux_loss
